# revision 2
# baseline (speedup 1.0000x reference)
"""TRN2 Bass kernel for CompressedLinearLayer: out = x @ (A @ B.T).T + bias.

Computed low-rank: t = x @ B  (rank 512), out = t @ A.T + bias.
Sharding: data-parallel over the 8192 rows of x (1024 rows per core);
B, A.T, bias replicated. No collectives.

Device layouts (per core):
  xT   [4096, 1024]  x rows shard, transposed on host (d_in on partitions)
  b    [4096, 512]   B as-is (d_in on partitions)
  at   [512, 4096]   A.T (rank on partitions)
  bias [4096]
  out  [1024, 4096]  natural orientation

Stage 1: tT[r, m] = sum_k B[k, r] * xT[k, m]     (rank on partitions)
Stage 2: out[m, d] = sum_r tT[r, m] * AT[r, d] + bias[d]

Matmuls run in float32r (TF32-like, full PE rate at N>=256, rel err ~1.5e-4).
"""
import numpy as np

import concourse.bacc as bacc
import concourse.mybir as mybir
import concourse.tile as tile
from concourse.bass_utils import run_bass_kernel_spmd

N_CORES = 8
BATCH, SEQ = 4, 2048
D_IN, D_OUT, RANK = 4096, 4096, 512
ROWS_TOTAL = BATCH * SEQ           # 8192
ROWS = ROWS_TOTAL // N_CORES       # 1024 rows per core

F32 = mybir.dt.float32
F32R = mybir.dt.float32r

KC = D_IN // 128     # 32 contraction chunks, stage 1
RC = RANK // 128     # 4 rank chunks
NB1 = ROWS // 512    # 2 row blocks of 512 (stage-1 moving dim)
MB2 = ROWS // 128    # 8 row chunks of 128 (stage-2 out partitions)
DB2 = D_OUT // 512   # 8 d_out blocks of 512 (stage-2 moving dim)

_compiled = {}


def _build():
    nc = bacc.Bacc("TRN2", target_bir_lowering=False, debug=False)

    xT_d = nc.declare_dram_parameter("xT", [D_IN, ROWS], F32R, isOutput=False)
    b_d = nc.declare_dram_parameter("b", [D_IN, RANK], F32R, isOutput=False)
    at_d = nc.declare_dram_parameter("at", [RANK, D_OUT], F32R, isOutput=False)
    bias_d = nc.declare_dram_parameter("bias", [D_OUT], F32, isOutput=False)
    out_d = nc.declare_dram_parameter("out", [ROWS, D_OUT], F32, isOutput=True)

    with tile.TileContext(nc) as tc:
        with (
            tc.tile_pool(name="wb", bufs=1) as wb,
            tc.tile_pool(name="xp", bufs=3) as xp,
            tc.tile_pool(name="tt", bufs=1) as ttp,
            tc.tile_pool(name="op", bufs=4) as op,
            tc.tile_pool(name="ps", bufs=8, space="PSUM") as ps,
        ):
            # B resident: 32 tiles [128, 512] (64KB/partition total)
            b_sb = []
            for k in range(KC):
                t = wb.tile([128, RANK], F32R, tag=f"b{k}", name=f"b{k}")
                nc.sync.dma_start(t[:], b_d[k * 128:(k + 1) * 128, :])
                b_sb.append(t)

            # bias broadcast to all partitions: [128, 4096]
            bias_bc = wb.tile([128, D_OUT], F32, tag="bias_bc")
            nc.sync.dma_start(bias_bc[0:1, :], bias_d[None, :])
            nc.gpsimd.partition_broadcast(bias_bc[:], bias_bc[0:1, :])

            # tT resident: 4 tiles [128, 1024] f32r
            tT = [ttp.tile([128, ROWS], F32R, tag=f"tT{r}", name=f"tT{r}") for r in range(RC)]

            # ---- Stage 1: tT = B.T @ xT (contraction over d_in) ----
            psum1 = [ps.tile([128, 512], F32, tag="ps", name=f"ps1_{i}") for i in range(RC * NB1)]
            for k in range(KC):
                xk = xp.tile([128, ROWS], F32R, tag="xk")
                nc.sync.dma_start(xk[:], xT_d[k * 128:(k + 1) * 128, :])
                for mc in range(RC):
                    for nb in range(NB1):
                        nc.tensor.matmul(
                            psum1[mc * NB1 + nb][:],
                            b_sb[k][:, mc * 128:(mc + 1) * 128],
                            xk[:, nb * 512:(nb + 1) * 512],
                            start=(k == 0),
                            stop=(k == KC - 1),
                        )
            for mc in range(RC):
                for nb in range(NB1):
                    nc.vector.tensor_copy(
                        tT[mc][:, nb * 512:(nb + 1) * 512], psum1[mc * NB1 + nb][:]
                    )

            # A.T resident: 4 tiles [128, 4096] (loaded late; needed for stage 2)
            at_sb = []
            for r in range(RC):
                t = wb.tile([128, D_OUT], F32R, tag=f"at{r}", name=f"at{r}")
                nc.sync.dma_start(t[:], at_d[r * 128:(r + 1) * 128, :])
                at_sb.append(t)

            # ---- Stage 2: out = t @ A.T + bias (contraction over rank) ----
            for rc2 in range(MB2):
                psum2 = [ps.tile([128, 512], F32, tag="ps", name=f"ps2_{rc2}_{i}") for i in range(DB2)]
                for k in range(RC):
                    for dc in range(DB2):
                        nc.tensor.matmul(
                            psum2[dc][:],
                            tT[k][:, rc2 * 128:(rc2 + 1) * 128],
                            at_sb[k][:, dc * 512:(dc + 1) * 512],
                            start=(k == 0),
                            stop=(k == RC - 1),
                        )
                for dc in range(DB2):
                    ot = op.tile([128, 512], F32, tag="ot")
                    nc.vector.tensor_add(
                        ot[:], psum2[dc][:], bias_bc[:, dc * 512:(dc + 1) * 512]
                    )
                    nc.sync.dma_start(
                        out_d[rc2 * 128:(rc2 + 1) * 128, dc * 512:(dc + 1) * 512],
                        ot[:],
                    )

    nc.compile()
    return nc


def _get_nc():
    if "nc" not in _compiled:
        _compiled["nc"] = _build()
    return _compiled["nc"]


def run(inputs, trace=False, trace_kwargs=None):
    """Shard, execute on 8 cores, gather. Returns (output, BassKernelResults)."""
    x = np.asarray(inputs["x"], dtype=np.float32)
    A = np.asarray(inputs["A"], dtype=np.float32)
    B = np.asarray(inputs["B"], dtype=np.float32)
    bias = np.asarray(inputs["bias"], dtype=np.float32)

    x_flat = x.reshape(ROWS_TOTAL, D_IN)
    AT = np.ascontiguousarray(A.T)
    in_maps = []
    for i in range(N_CORES):
        xT_i = np.ascontiguousarray(x_flat[i * ROWS:(i + 1) * ROWS].T)
        in_maps.append({"xT": xT_i, "b": B, "at": AT, "bias": bias})

    nc = _get_nc()
    kwargs = {}
    if trace:
        kwargs["trace"] = True
        kwargs["trace_kwargs"] = trace_kwargs or {}
    res = run_bass_kernel_spmd(nc, in_maps, core_ids=list(range(N_CORES)), **kwargs)

    out = np.concatenate([res.results[i]["out"] for i in range(N_CORES)], axis=0)
    return out.reshape(BATCH, SEQ, D_OUT), res


def kernel(**inputs) -> np.ndarray:
    out, _ = run(inputs)
    return out
